# revision 50
# baseline (speedup 1.0000x reference)
"""Trainium2 Bass kernel for the concept-embedding model (CEM) dense MLP.

Reference computation (B=2048, C=64, D=1024, E=128):
    h      = relu(x @ W1[c] + b1[c])                 [B,C,D]
    c_pred = sigmoid(h . w2[c] + b2[c])              [B,C]
    xc     = [x ; c_pred]                            [B,C,D+1]
    emb    = relu(xc @ Wl[c] + bl[c])                [B,C,D+1]
    mu     = emb @ Wmu + bmu                         [B,C,E]
    logvar = emb @ Wlv + blv                         [B,C,E]
    (mask == 0 in eval mode, so c_emb == mu and c_int/proto_* are unused)

Strategy: expert(concept)-parallel over 8 NeuronCores, 8 concepts per core,
no collectives (outputs gathered on host). All activations live in
[feature, batch] layout so consecutive GEMMs chain on the TensorEngine with
no transposes; x is transposed once on the host. Compute in bf16 with fp32
PSUM accumulation; outputs in fp32.
"""

import numpy as np
import ml_dtypes

B, C, D, E = 2048, 64, 1024, 128
N_CORES = 8
CL = C // N_CORES          # concepts per core
PB = 512                   # batch tile (PSUM bank limit for f32)
NB = B // PB               # 4 batch tiles
KT = D // 128              # 8 k-tiles over D
JT = 9                     # j-tiles over D+1 padded to 1152
JP = JT * 128              # 1152

BF16 = ml_dtypes.bfloat16
FP8 = ml_dtypes.float8_e4m3
FP8_G1 = True              # GEMM1 (x@W1) in fp8-e4m3 DoubleRow, 2x PE rate
W1_SCALE = 16.0            # pre-scale W1 into e4m3 normal range

_COMPILED = None           # (nc, names) cache — compile once per process


def _build():
    import concourse.bass as bass  # noqa: F401  (registers engine methods)
    import concourse.mybir as mybir
    import concourse.tile as tile
    from concourse import bacc, bass_isa

    f32 = mybir.dt.float32
    bf16 = mybir.dt.bfloat16
    fp8 = mybir.dt.float8e4
    g1dt = fp8 if FP8_G1 else bf16
    AF = mybir.ActivationFunctionType
    ALU = mybir.AluOpType

    nc = bacc.Bacc("TRN2", target_bir_lowering=False, debug=False,
                   num_devices=N_CORES)

    xt_d = nc.dram_tensor("xt", [NB, D, PB], bf16, kind="ExternalInput")
    if FP8_G1:
        xt8_d = nc.dram_tensor("xt8", [NB, D, PB], fp8, kind="ExternalInput")
    w1_d = nc.dram_tensor("w1", [CL, D, D], g1dt, kind="ExternalInput")
    b1_d = nc.dram_tensor("b1t", [CL, 128, KT], f32, kind="ExternalInput")
    w2_d = nc.dram_tensor("w2t", [CL, 128, KT], f32, kind="ExternalInput")
    b2_d = nc.dram_tensor("b2", [1, CL], f32, kind="ExternalInput")
    wl_d = nc.dram_tensor("wl", [CL, D, JP], bf16, kind="ExternalInput")
    wlr_d = nc.dram_tensor("wlrt", [CL, 128, JT], f32, kind="ExternalInput")
    bl_d = nc.dram_tensor("blt", [CL, 128, JT], f32, kind="ExternalInput")
    wmu_d = nc.dram_tensor("wmu", [128, KT, E], bf16, kind="ExternalInput")
    bmu_d = nc.dram_tensor("bmu", [E, 1], f32, kind="ExternalInput")
    wmur_d = nc.dram_tensor("wmur", [E, 1], f32, kind="ExternalInput")
    wlv_d = nc.dram_tensor("wlv", [128, KT, E], bf16, kind="ExternalInput")
    blv_d = nc.dram_tensor("blv", [E, 1], f32, kind="ExternalInput")
    wlvr_d = nc.dram_tensor("wlvr", [E, 1], f32, kind="ExternalInput")

    cp_o = nc.dram_tensor("c_pred", [CL, B], f32, kind="ExternalOutput")
    mu_o = nc.dram_tensor("mu", [CL, E, B], f32, kind="ExternalOutput")
    lv_o = nc.dram_tensor("lv", [CL, E, B], f32, kind="ExternalOutput")

    with tile.TileContext(nc) as tc:
        with (
            tc.tile_pool(name="xp", bufs=1) as xp,
            tc.tile_pool(name="const", bufs=1) as constp,
            tc.tile_pool(name="w1p", bufs=2) as w1p,
            tc.tile_pool(name="wlp", bufs=2) as wlp,
            tc.tile_pool(name="cb", bufs=2) as cbp,
            tc.tile_pool(name="hp", bufs=3) as hp,
            tc.tile_pool(name="ep", bufs=2) as ep,
            tc.tile_pool(name="cpp", bufs=5) as cpp,
            tc.tile_pool(name="rp", bufs=3) as rp,
            tc.tile_pool(name="op", bufs=4) as op,
            tc.tile_pool(name="acc", bufs=6, space="PSUM") as accp,
            tc.tile_pool(name="psh", bufs=2, space="PSUM") as pshp,
        ):
            # ---- resident tensors ----
            # xt loads split per (k, b-tile), first b-tile first, so the PE
            # can start as soon as the first slices land
            # DMA issue costs ~0.6us/instr on one sequencer, but one DMA
            # rides one queue (~64-100GB/s): chunk big loads over a few
            # queues; round-robin cold-start issues over idle sequencers.
            _eng = [nc.sync, nc.scalar, nc.gpsimd]
            _ei = [0]

            def _issue(out_ap, in_ap, spread):
                e = _eng[_ei[0] % len(_eng)] if spread else nc.sync
                _ei[0] += 1
                e.dma_start(out_ap, in_ap)

            def load_x(tile, dram, bi, chunks=2, spread=False):
                kc = KT // chunks
                for c in range(chunks):
                    _issue(
                        tile[:, c * kc:(c + 1) * kc, bi * PB:(bi + 1) * PB],
                        dram[bi, c * kc * 128:(c + 1) * kc * 128, :]
                        .rearrange("(k p) n -> p k n", p=128), spread)

            def load_w(tile, dram_ci, chunks, spread=False):
                if chunks <= KT:
                    kc = KT // chunks
                    for c in range(chunks):
                        _issue(
                            tile[:, c * kc:(c + 1) * kc, :],
                            dram_ci[c * kc * 128:(c + 1) * kc * 128, :]
                            .rearrange("(k p) d -> p k d", p=128), spread)
                else:
                    # split each k-slab along the free dim too
                    jc = dram_ci.shape[1] // (chunks // KT)
                    for k in range(KT):
                        for c in range(chunks // KT):
                            _issue(
                                tile[:, k, c * jc:(c + 1) * jc],
                                dram_ci[k * 128:(k + 1) * 128,
                                        c * jc:(c + 1) * jc], spread)

            xt = xp.tile([128, KT, B], bf16)
            if FP8_G1:
                xt8 = xp.tile([128, KT, B], fp8, tag="xt8")
                load_x(xt8, xt8_d, 0, chunks=4, spread=True)
            else:
                load_x(xt, xt_d, 0, chunks=4, spread=True)
            w1c0 = w1p.tile([128, KT, D], g1dt, tag="w1c")
            load_w(w1c0, w1_d[0], 4, spread=True)
            if FP8_G1:
                load_x(xt8, xt8_d, 1, chunks=2, spread=True)
                load_x(xt, xt_d, 0, chunks=4, spread=True)
            wlc0 = wlp.tile([128, KT, JP], bf16, tag="wlc")
            load_w(wlc0, wl_d[0], 8, spread=True)
            for bi in range(1, NB):
                if FP8_G1:
                    if bi > 1:
                        load_x(xt8, xt8_d, bi)
                else:
                    load_x(xt, xt_d, bi, spread=(bi == 1))
                if FP8_G1:
                    load_x(xt, xt_d, bi, spread=(bi == 1))
            wmu = constp.tile([128, KT, E], bf16, tag="wmu")
            nc.sync.dma_start(wmu[:], wmu_d[:])
            wlv = constp.tile([128, KT, E], bf16, tag="wlv")
            nc.sync.dma_start(wlv[:], wlv_d[:])
            bmu = constp.tile([E, 1], f32, tag="bmu")
            nc.sync.dma_start(bmu[:], bmu_d[:])
            blv = constp.tile([E, 1], f32, tag="blv")
            nc.sync.dma_start(blv[:], blv_d[:])
            wmur = constp.tile([E, 1], f32, tag="wmur")
            nc.sync.dma_start(wmur[:], wmur_d[:])
            wlvr = constp.tile([E, 1], f32, tag="wlvr")
            nc.sync.dma_start(wlvr[:], wlvr_d[:])
            b2s = constp.tile([1, CL], f32, tag="b2s")
            nc.sync.dma_start(b2s[:], b2_d[:])

            for ci in range(CL):
                # ---- per-concept weights (double-buffered) ----
                if ci == 0:
                    w1c = w1c0
                    wlc = wlc0
                else:
                    w1c = w1p.tile([128, KT, D], g1dt, tag="w1c")
                    load_w(w1c, w1_d[ci], 2)
                    wlc = wlp.tile([128, KT, JP], bf16, tag="wlc")
                    load_w(wlc, wl_d[ci], 4)
                wlr = cbp.tile([128, JT], f32, tag="wlr")
                nc.sync.dma_start(wlr[:], wlr_d[ci])
                b1c = cbp.tile([128, KT], f32, tag="b1c")
                nc.sync.dma_start(b1c[:], b1_d[ci])
                w2c = cbp.tile([128, KT], f32, tag="w2c")
                nc.sync.dma_start(w2c[:], w2_d[ci])
                blc = cbp.tile([128, JT], f32, tag="blc")
                nc.sync.dma_start(blc[:], bl_d[ci])

                def phase1(bi, w1c=w1c, b1c=b1c, w2c=w2c, ci=ci):
                    bsl = slice(bi * PB, (bi + 1) * PB)
                    # ---- GEMM1: hT = relu(W1c^T x) ; s += w2c . hT ----
                    # scorer reduction stays off the PE: DVE accumulates
                    # hth*w2 over m-tiles, GpSimd reduces across partitions
                    hw = None
                    for m in range(KT):
                        acc = accp.tile([128, PB], f32, tag="acc")
                        if FP8_G1:
                            for k2 in range(0, KT, 2):
                                nc.tensor.matmul(
                                    acc[:],
                                    w1c[:, k2:k2 + 2, m * 128:(m + 1) * 128],
                                    xt8[:, k2:k2 + 2, bsl],
                                    start=(k2 == 0), stop=(k2 == KT - 2),
                                    perf_mode=mybir.MatmulPerfMode.DoubleRow,
                                )
                        else:
                            for k in range(KT):
                                nc.tensor.matmul(
                                    acc[:],
                                    w1c[:, k, m * 128:(m + 1) * 128],
                                    xt[:, k, bsl],
                                    start=(k == 0), stop=(k == KT - 1),
                                )
                        hth = hp.tile([128, PB], bf16)
                        nc.scalar.activation(hth[:], acc[:], AF.Relu,
                                             bias=b1c[:, m:m + 1],
                                             scale=(1.0 / W1_SCALE)
                                             if FP8_G1 else 1.0)
                        hw2 = rp.tile([128, PB], f32, tag="hw")
                        if m == 0:
                            nc.vector.tensor_scalar(
                                hw2[:], hth[:], w2c[:, m:m + 1], None,
                                ALU.mult)
                        else:
                            nc.vector.scalar_tensor_tensor(
                                hw2[:], hth[:], w2c[:, m:m + 1], hw[:],
                                ALU.mult, ALU.add)
                        hw = hw2
                    sred = rp.tile([128, PB], f32, tag="sred")
                    nc.gpsimd.partition_all_reduce(sred[:], hw[:], 128,
                                                   bass_isa.ReduceOp.add)
                    # ---- c_pred = sigmoid(s + b2) ----
                    cpf = cpp.tile([1, PB], f32, tag="cpf")
                    nc.scalar.activation(cpf[:], sred[0:1, :], AF.Sigmoid,
                                         bias=b2s[0:1, ci:ci + 1])
                    cpb = cpp.tile([1, PB], bf16, tag="cpb")
                    nc.vector.tensor_copy(cpb[:], cpf[:])
                    nc.sync.dma_start(cp_o[ci:ci + 1, bsl], cpf[:])
                    # broadcast cp to all 128 partitions (GpSimd)
                    cpB = cpp.tile([128, PB], bf16, tag="cpB")
                    nc.gpsimd.partition_broadcast(cpB[:], cpb[:])
                    return cpB

                def phase2(bi, cpB, wlc=wlc, wlr=wlr, blc=blc, ci=ci):
                    bsl = slice(bi * PB, (bi + 1) * PB)
                    # ---- GEMM2: embT = relu(Wl^T x + wlrow*cp + bl) ----
                    emb = ep.tile([128, JT, PB], bf16)
                    for j in range(JT):
                        acc2 = accp.tile([128, PB], f32, tag="acc")
                        for k in range(KT):
                            nc.tensor.matmul(
                                acc2[:],
                                wlc[:, k, j * 128:(j + 1) * 128],
                                xt[:, k, bsl],
                                start=(k == 0), stop=(k == KT - 1),
                            )
                        # t2 = cp * wlrow_j + psum  (rank-1 cp column fold)
                        t2 = rp.tile([128, PB], f32, tag="t2")
                        nc.vector.scalar_tensor_tensor(
                            t2[:], cpB[:], wlr[:, j:j + 1], acc2[:],
                            ALU.mult, ALU.add)
                        nc.scalar.activation(emb[:, j, :], t2[:], AF.Relu,
                                             bias=blc[:, j:j + 1])
                    # ---- heads: mu / logvar ----
                    # j=1024 row handled as a rank-1 DVE fold (e1024 bcast)
                    e1024B = cpp.tile([128, PB], bf16, tag="e1024B")
                    nc.gpsimd.partition_broadcast(e1024B[:], emb[0:1, KT, :])
                    for wt, wr, bias, outd in ((wmu, wmur, bmu, mu_o),
                                               (wlv, wlvr, blv, lv_o)):
                        acch = pshp.tile([E, PB], f32)
                        for j in range(KT):
                            nc.tensor.matmul(acch[:], wt[:, j, :],
                                             emb[:, j, :],
                                             start=(j == 0), stop=(j == KT - 1))
                        t3 = rp.tile([E, PB], f32, tag="t3")
                        nc.vector.scalar_tensor_tensor(
                            t3[:], e1024B[:], wr[:, 0:1], acch[:],
                            ALU.mult, ALU.add)
                        oth = op.tile([E, PB], f32)
                        nc.scalar.activation(oth[:], t3[:], AF.Identity,
                                             bias=bias[:, 0:1])
                        nc.sync.dma_start(outd[ci, :, bsl], oth[:])

                if ci == 0:
                    # cold start: skew phase1 two b-tiles ahead so the
                    # GEMM2 operand DMAs (xt bf16 + wlc) have time to land
                    cpB0 = phase1(0)
                    cpB1 = phase1(1)
                    phase2(0, cpB0)
                    cpB2 = phase1(2)
                    phase2(1, cpB1)
                    cpB3 = phase1(3)
                    phase2(2, cpB2)
                    phase2(3, cpB3)
                else:
                    for bi in range(NB):
                        phase2(bi, phase1(bi))

    nc.compile()
    return nc


def _get_compiled():
    global _COMPILED
    if _COMPILED is None:
        _COMPILED = _build()
    return _COMPILED


def kernel(x, c_int, W1, b1, w2, b2, Wl, bl, Wmu, bmu, Wlv, blv,
           proto_pos, proto_neg):
    from concourse.bass_utils import run_bass_kernel_spmd

    x = np.asarray(x, dtype=np.float32)
    W1 = np.asarray(W1, dtype=np.float32)
    b1 = np.asarray(b1, dtype=np.float32)
    w2 = np.asarray(w2, dtype=np.float32)
    b2 = np.asarray(b2, dtype=np.float32)
    Wl = np.asarray(Wl, dtype=np.float32)
    bl = np.asarray(bl, dtype=np.float32)
    Wmu = np.asarray(Wmu, dtype=np.float32)
    bmu = np.asarray(bmu, dtype=np.float32)
    Wlv = np.asarray(Wlv, dtype=np.float32)
    blv = np.asarray(blv, dtype=np.float32)

    # ---- host-side prep: transpose/pad/retile, cast to bf16 ----
    # x stored b-tile-sliced [NB, D, PB] so each (k, b) SBUF slice is one
    # contiguous 128KB DMA
    xTb = np.ascontiguousarray(x.T.reshape(D, NB, PB).transpose(1, 0, 2))
    xt = xTb.astype(BF16)
    if FP8_G1:
        xt8 = xTb.astype(FP8)
        w1_bf = (W1 * W1_SCALE).astype(FP8)
    else:
        w1_bf = W1.astype(BF16)                                    # [C, D, D]
    b1t = np.ascontiguousarray(
        b1.reshape(C, KT, 128).transpose(0, 2, 1)).astype(np.float32)
    w2t = np.ascontiguousarray(
        w2.reshape(C, KT, 128).transpose(0, 2, 1)).astype(np.float32)
    wl_pad = np.zeros((C, D, JP), dtype=BF16)
    wl_pad[:, :, :D + 1] = Wl[:, :D, :].astype(BF16)
    wlrow = np.zeros((C, JP), dtype=np.float32)
    wlrow[:, :D + 1] = Wl[:, D, :]
    wlrt = np.ascontiguousarray(wlrow.reshape(C, JT, 128).transpose(0, 2, 1))
    bl_pad = np.zeros((C, JP), dtype=np.float32)
    bl_pad[:, :D + 1] = bl
    blt = np.ascontiguousarray(bl_pad.reshape(C, JT, 128).transpose(0, 2, 1))
    wmu_t = np.ascontiguousarray(
        Wmu[:D].astype(BF16).reshape(KT, 128, E).transpose(1, 0, 2))
    wlv_t = np.ascontiguousarray(
        Wlv[:D].astype(BF16).reshape(KT, 128, E).transpose(1, 0, 2))
    wmur = np.ascontiguousarray(Wmu[D].reshape(E, 1)).astype(np.float32)
    wlvr = np.ascontiguousarray(Wlv[D].reshape(E, 1)).astype(np.float32)
    bmu2 = bmu.reshape(E, 1).astype(np.float32)
    blv2 = blv.reshape(E, 1).astype(np.float32)

    in_maps = []
    for c in range(N_CORES):
        sl = slice(c * CL, (c + 1) * CL)
        im = {
            "xt": xt,
            "w1": w1_bf[sl],
            "b1t": b1t[sl],
            "w2t": w2t[sl],
            "b2": np.ascontiguousarray(b2[sl].reshape(1, CL)),
            "wl": wl_pad[sl],
            "wlrt": wlrt[sl],
            "blt": blt[sl],
            "wmu": wmu_t,
            "bmu": bmu2,
            "wmur": wmur,
            "wlv": wlv_t,
            "blv": blv2,
            "wlvr": wlvr,
        }
        if FP8_G1:
            im["xt8"] = xt8
        in_maps.append(im)

    nc = _get_compiled()
    res = run_bass_kernel_spmd(nc, in_maps, core_ids=list(range(N_CORES)))
    kernel.last_results = res

    c_pred = np.concatenate([res.results[c]["c_pred"] for c in range(N_CORES)],
                            axis=0).T.astype(np.float32)            # [B, C]
    mu = np.concatenate([res.results[c]["mu"] for c in range(N_CORES)],
                        axis=0).transpose(2, 0, 1).astype(np.float32)  # [B,C,E]
    logvar = np.concatenate([res.results[c]["lv"] for c in range(N_CORES)],
                            axis=0).transpose(2, 0, 1).astype(np.float32)
    c_emb = mu.copy()
    return (np.ascontiguousarray(c_pred), c_emb, mu,
            np.ascontiguousarray(logvar))


# revision 52
# speedup vs baseline: 1.2112x; 1.2112x over previous
"""Trainium2 Bass kernel for the concept-embedding model (CEM) dense MLP.

Reference computation (B=2048, C=64, D=1024, E=128):
    h      = relu(x @ W1[c] + b1[c])                 [B,C,D]
    c_pred = sigmoid(h . w2[c] + b2[c])              [B,C]
    xc     = [x ; c_pred]                            [B,C,D+1]
    emb    = relu(xc @ Wl[c] + bl[c])                [B,C,D+1]
    mu     = emb @ Wmu + bmu                         [B,C,E]
    logvar = emb @ Wlv + blv                         [B,C,E]
    (mask == 0 in eval mode, so c_emb == mu and c_int/proto_* are unused)

Strategy: expert(concept)-parallel over 8 NeuronCores, 8 concepts per core,
no collectives (outputs gathered on host). All activations live in
[feature, batch] layout so consecutive GEMMs chain on the TensorEngine with
no transposes; x is transposed once on the host. Compute in bf16 with fp32
PSUM accumulation; outputs in fp32.
"""

import numpy as np
import ml_dtypes

B, C, D, E = 2048, 64, 1024, 128
N_CORES = 8
CL = C // N_CORES          # concepts per core
PB = 512                   # batch tile (PSUM bank limit for f32)
NB = B // PB               # 4 batch tiles
KT = D // 128              # 8 k-tiles over D
JT = 9                     # j-tiles over D+1 padded to 1152
JP = JT * 128              # 1152

BF16 = ml_dtypes.bfloat16
FP8 = ml_dtypes.float8_e4m3
FP8_G1 = True              # GEMM1 (x@W1) in fp8-e4m3 DoubleRow, 2x PE rate
W1_SCALE = 16.0            # pre-scale W1 into e4m3 normal range

_COMPILED = None           # (nc, names) cache — compile once per process


def _build():
    import concourse.bass as bass  # noqa: F401  (registers engine methods)
    import concourse.mybir as mybir
    import concourse.tile as tile
    from concourse import bacc, bass_isa

    f32 = mybir.dt.float32
    bf16 = mybir.dt.bfloat16
    fp8 = mybir.dt.float8e4
    g1dt = fp8 if FP8_G1 else bf16
    AF = mybir.ActivationFunctionType
    ALU = mybir.AluOpType

    nc = bacc.Bacc("TRN2", target_bir_lowering=False, debug=False,
                   num_devices=N_CORES)

    xt_d = nc.dram_tensor("xt", [NB, D, PB], bf16, kind="ExternalInput")
    if FP8_G1:
        xt8_d = nc.dram_tensor("xt8", [NB, D, PB], fp8, kind="ExternalInput")
    w1_d = nc.dram_tensor("w1", [CL, D, D], g1dt, kind="ExternalInput")
    b1_d = nc.dram_tensor("b1t", [CL, 128, KT], f32, kind="ExternalInput")
    w2_d = nc.dram_tensor("w2t", [CL, 128, KT], f32, kind="ExternalInput")
    b2_d = nc.dram_tensor("b2", [1, CL], f32, kind="ExternalInput")
    wl_d = nc.dram_tensor("wl", [CL, D, JP], bf16, kind="ExternalInput")
    wlr_d = nc.dram_tensor("wlrt", [CL, 128, JT], f32, kind="ExternalInput")
    bl_d = nc.dram_tensor("blt", [CL, 128, JT], f32, kind="ExternalInput")
    wmu_d = nc.dram_tensor("wmu", [128, KT, E], bf16, kind="ExternalInput")
    bmu_d = nc.dram_tensor("bmu", [E, 1], f32, kind="ExternalInput")
    wmur_d = nc.dram_tensor("wmur", [E, 1], f32, kind="ExternalInput")
    wlv_d = nc.dram_tensor("wlv", [128, KT, E], bf16, kind="ExternalInput")
    blv_d = nc.dram_tensor("blv", [E, 1], f32, kind="ExternalInput")
    wlvr_d = nc.dram_tensor("wlvr", [E, 1], f32, kind="ExternalInput")

    cp_o = nc.dram_tensor("c_pred", [CL, B], f32, kind="ExternalOutput")
    mu_o = nc.dram_tensor("mu", [CL, E, B], f32, kind="ExternalOutput")
    lv_o = nc.dram_tensor("lv", [CL, E, B], f32, kind="ExternalOutput")

    with tile.TileContext(nc) as tc:
        with (
            tc.tile_pool(name="xp", bufs=1) as xp,
            tc.tile_pool(name="const", bufs=1) as constp,
            tc.tile_pool(name="w1p", bufs=2) as w1p,
            tc.tile_pool(name="wlp", bufs=2) as wlp,
            tc.tile_pool(name="cb", bufs=2) as cbp,
            tc.tile_pool(name="hp", bufs=3) as hp,
            tc.tile_pool(name="ep", bufs=2) as ep,
            tc.tile_pool(name="cpp", bufs=5) as cpp,
            tc.tile_pool(name="rp", bufs=3) as rp,
            tc.tile_pool(name="op", bufs=4) as op,
            tc.tile_pool(name="acc", bufs=6, space="PSUM") as accp,
            tc.tile_pool(name="psh", bufs=2, space="PSUM") as pshp,
        ):
            # ---- resident tensors ----
            # xt loads split per (k, b-tile), first b-tile first, so the PE
            # can start as soon as the first slices land
            # DMA issue costs ~0.6us/instr on one sequencer, but one DMA
            # rides one queue (~64-100GB/s): chunk big loads over a few
            # queues; round-robin cold-start issues over idle sequencers.
            _eng = [nc.sync, nc.scalar, nc.gpsimd]
            _ei = [0]

            def _issue(out_ap, in_ap, spread):
                e = _eng[_ei[0] % len(_eng)] if spread else nc.sync
                _ei[0] += 1
                e.dma_start(out_ap, in_ap)

            def load_x(tile, dram, bi, chunks=2, spread=False):
                kc = KT // chunks
                for c in range(chunks):
                    _issue(
                        tile[:, c * kc:(c + 1) * kc, bi * PB:(bi + 1) * PB],
                        dram[bi, c * kc * 128:(c + 1) * kc * 128, :]
                        .rearrange("(k p) n -> p k n", p=128), spread)

            def load_w(tile, dram_ci, chunks, spread=False):
                if chunks <= KT:
                    kc = KT // chunks
                    for c in range(chunks):
                        _issue(
                            tile[:, c * kc:(c + 1) * kc, :],
                            dram_ci[c * kc * 128:(c + 1) * kc * 128, :]
                            .rearrange("(k p) d -> p k d", p=128), spread)
                else:
                    # split each k-slab along the free dim too
                    jc = dram_ci.shape[1] // (chunks // KT)
                    for k in range(KT):
                        for c in range(chunks // KT):
                            _issue(
                                tile[:, k, c * jc:(c + 1) * jc],
                                dram_ci[k * 128:(k + 1) * 128,
                                        c * jc:(c + 1) * jc], spread)

            # tiny biases/constants first — they gate the first evictions
            b1c0 = cbp.tile([128, KT], f32, tag="b1c")
            nc.sync.dma_start(b1c0[:], b1_d[0])
            w2c0 = cbp.tile([128, KT], f32, tag="w2c")
            nc.scalar.dma_start(w2c0[:], w2_d[0])
            wlr0 = cbp.tile([128, JT], f32, tag="wlr")
            nc.gpsimd.dma_start(wlr0[:], wlr_d[0])
            blc0 = cbp.tile([128, JT], f32, tag="blc")
            nc.sync.dma_start(blc0[:], bl_d[0])
            b2s = constp.tile([1, CL], f32, tag="b2s")
            nc.scalar.dma_start(b2s[:], b2_d[:])
            bmu = constp.tile([E, 1], f32, tag="bmu")
            nc.gpsimd.dma_start(bmu[:], bmu_d[:])
            blv = constp.tile([E, 1], f32, tag="blv")
            nc.sync.dma_start(blv[:], blv_d[:])
            wmur = constp.tile([E, 1], f32, tag="wmur")
            nc.scalar.dma_start(wmur[:], wmur_d[:])
            wlvr = constp.tile([E, 1], f32, tag="wlvr")
            nc.gpsimd.dma_start(wlvr[:], wlvr_d[:])

            xt = xp.tile([128, KT, B], bf16)
            if FP8_G1:
                xt8 = xp.tile([128, KT, B], fp8, tag="xt8")
                load_x(xt8, xt8_d, 0, chunks=4, spread=True)
            else:
                load_x(xt, xt_d, 0, chunks=4, spread=True)
            w1c0 = w1p.tile([128, KT, D], g1dt, tag="w1c")
            load_w(w1c0, w1_d[0], 4, spread=True)
            if FP8_G1:
                load_x(xt8, xt8_d, 1, chunks=2, spread=True)
                load_x(xt, xt_d, 0, chunks=4, spread=True)
            wlc0 = wlp.tile([128, KT, JP], bf16, tag="wlc")
            load_w(wlc0, wl_d[0], 8, spread=True)
            wmu = constp.tile([128, KT, E], bf16, tag="wmu")
            nc.scalar.dma_start(wmu[:], wmu_d[:])
            wlv = constp.tile([128, KT, E], bf16, tag="wlv")
            nc.gpsimd.dma_start(wlv[:], wlv_d[:])
            for bi in range(1, NB):
                if FP8_G1:
                    if bi > 1:
                        load_x(xt8, xt8_d, bi)
                else:
                    load_x(xt, xt_d, bi, spread=(bi == 1))
                if FP8_G1:
                    load_x(xt, xt_d, bi, spread=(bi == 1))

            for ci in range(CL):
                # ---- per-concept weights (double-buffered) ----
                if ci == 0:
                    w1c, wlc = w1c0, wlc0
                    wlr, b1c, w2c, blc = wlr0, b1c0, w2c0, blc0
                else:
                    wlr = cbp.tile([128, JT], f32, tag="wlr")
                    nc.sync.dma_start(wlr[:], wlr_d[ci])
                    b1c = cbp.tile([128, KT], f32, tag="b1c")
                    nc.sync.dma_start(b1c[:], b1_d[ci])
                    w2c = cbp.tile([128, KT], f32, tag="w2c")
                    nc.sync.dma_start(w2c[:], w2_d[ci])
                    blc = cbp.tile([128, JT], f32, tag="blc")
                    nc.sync.dma_start(blc[:], bl_d[ci])
                    w1c = w1p.tile([128, KT, D], g1dt, tag="w1c")
                    load_w(w1c, w1_d[ci], 2)
                    wlc = wlp.tile([128, KT, JP], bf16, tag="wlc")
                    load_w(wlc, wl_d[ci], 4)

                def phase1(bi, w1c=w1c, b1c=b1c, w2c=w2c, ci=ci):
                    bsl = slice(bi * PB, (bi + 1) * PB)
                    # ---- GEMM1: hT = relu(W1c^T x) ; s += w2c . hT ----
                    # scorer reduction stays off the PE: DVE accumulates
                    # hth*w2 over m-tiles, GpSimd reduces across partitions
                    hw = None
                    for m in range(KT):
                        acc = accp.tile([128, PB], f32, tag="acc")
                        if FP8_G1:
                            for k2 in range(0, KT, 2):
                                nc.tensor.matmul(
                                    acc[:],
                                    w1c[:, k2:k2 + 2, m * 128:(m + 1) * 128],
                                    xt8[:, k2:k2 + 2, bsl],
                                    start=(k2 == 0), stop=(k2 == KT - 2),
                                    perf_mode=mybir.MatmulPerfMode.DoubleRow,
                                )
                        else:
                            for k in range(KT):
                                nc.tensor.matmul(
                                    acc[:],
                                    w1c[:, k, m * 128:(m + 1) * 128],
                                    xt[:, k, bsl],
                                    start=(k == 0), stop=(k == KT - 1),
                                )
                        hth = hp.tile([128, PB], bf16)
                        nc.scalar.activation(hth[:], acc[:], AF.Relu,
                                             bias=b1c[:, m:m + 1],
                                             scale=(1.0 / W1_SCALE)
                                             if FP8_G1 else 1.0)
                        hw2 = rp.tile([128, PB], f32, tag="hw")
                        if m == 0:
                            nc.vector.tensor_scalar(
                                hw2[:], hth[:], w2c[:, m:m + 1], None,
                                ALU.mult)
                        else:
                            nc.vector.scalar_tensor_tensor(
                                hw2[:], hth[:], w2c[:, m:m + 1], hw[:],
                                ALU.mult, ALU.add)
                        hw = hw2
                    sred = rp.tile([128, PB], f32, tag="sred")
                    nc.gpsimd.partition_all_reduce(sred[:], hw[:], 128,
                                                   bass_isa.ReduceOp.add)
                    # ---- c_pred = sigmoid(s + b2) ----
                    cpf = cpp.tile([1, PB], f32, tag="cpf")
                    nc.scalar.activation(cpf[:], sred[0:1, :], AF.Sigmoid,
                                         bias=b2s[0:1, ci:ci + 1])
                    cpb = cpp.tile([1, PB], bf16, tag="cpb")
                    nc.vector.tensor_copy(cpb[:], cpf[:])
                    nc.sync.dma_start(cp_o[ci:ci + 1, bsl], cpf[:])
                    # broadcast cp to all 128 partitions (GpSimd)
                    cpB = cpp.tile([128, PB], bf16, tag="cpB")
                    nc.gpsimd.partition_broadcast(cpB[:], cpb[:])
                    return cpB

                def phase2(bi, cpB, wlc=wlc, wlr=wlr, blc=blc, ci=ci):
                    bsl = slice(bi * PB, (bi + 1) * PB)
                    # ---- GEMM2: embT = relu(Wl^T x + wlrow*cp + bl) ----
                    emb = ep.tile([128, JT, PB], bf16)
                    for j in range(JT):
                        acc2 = accp.tile([128, PB], f32, tag="acc")
                        for k in range(KT):
                            nc.tensor.matmul(
                                acc2[:],
                                wlc[:, k, j * 128:(j + 1) * 128],
                                xt[:, k, bsl],
                                start=(k == 0), stop=(k == KT - 1),
                            )
                        # t2 = cp * wlrow_j + psum  (rank-1 cp column fold)
                        t2 = rp.tile([128, PB], f32, tag="t2")
                        nc.vector.scalar_tensor_tensor(
                            t2[:], cpB[:], wlr[:, j:j + 1], acc2[:],
                            ALU.mult, ALU.add)
                        nc.scalar.activation(emb[:, j, :], t2[:], AF.Relu,
                                             bias=blc[:, j:j + 1])
                    # ---- heads: mu / logvar ----
                    # j=1024 row handled as a rank-1 DVE fold (e1024 bcast)
                    e1024B = cpp.tile([128, PB], bf16, tag="e1024B")
                    nc.gpsimd.partition_broadcast(e1024B[:], emb[0:1, KT, :])
                    for wt, wr, bias, outd in ((wmu, wmur, bmu, mu_o),
                                               (wlv, wlvr, blv, lv_o)):
                        acch = pshp.tile([E, PB], f32)
                        for j in range(KT):
                            nc.tensor.matmul(acch[:], wt[:, j, :],
                                             emb[:, j, :],
                                             start=(j == 0), stop=(j == KT - 1))
                        t3 = rp.tile([E, PB], f32, tag="t3")
                        nc.vector.scalar_tensor_tensor(
                            t3[:], e1024B[:], wr[:, 0:1], acch[:],
                            ALU.mult, ALU.add)
                        oth = op.tile([E, PB], f32)
                        nc.scalar.activation(oth[:], t3[:], AF.Identity,
                                             bias=bias[:, 0:1])
                        nc.sync.dma_start(outd[ci, :, bsl], oth[:])

                if ci == 0:
                    # cold start: skew phase1 two b-tiles ahead so the
                    # GEMM2 operand DMAs (xt bf16 + wlc) have time to land
                    cpB0 = phase1(0)
                    cpB1 = phase1(1)
                    phase2(0, cpB0)
                    cpB2 = phase1(2)
                    phase2(1, cpB1)
                    cpB3 = phase1(3)
                    phase2(2, cpB2)
                    phase2(3, cpB3)
                else:
                    for bi in range(NB):
                        phase2(bi, phase1(bi))

    nc.compile()
    return nc


def _get_compiled():
    global _COMPILED
    if _COMPILED is None:
        _COMPILED = _build()
    return _COMPILED


def kernel(x, c_int, W1, b1, w2, b2, Wl, bl, Wmu, bmu, Wlv, blv,
           proto_pos, proto_neg):
    from concourse.bass_utils import run_bass_kernel_spmd

    x = np.asarray(x, dtype=np.float32)
    W1 = np.asarray(W1, dtype=np.float32)
    b1 = np.asarray(b1, dtype=np.float32)
    w2 = np.asarray(w2, dtype=np.float32)
    b2 = np.asarray(b2, dtype=np.float32)
    Wl = np.asarray(Wl, dtype=np.float32)
    bl = np.asarray(bl, dtype=np.float32)
    Wmu = np.asarray(Wmu, dtype=np.float32)
    bmu = np.asarray(bmu, dtype=np.float32)
    Wlv = np.asarray(Wlv, dtype=np.float32)
    blv = np.asarray(blv, dtype=np.float32)

    # ---- host-side prep: transpose/pad/retile, cast to bf16 ----
    # x stored b-tile-sliced [NB, D, PB] so each (k, b) SBUF slice is one
    # contiguous 128KB DMA
    xTb = np.ascontiguousarray(x.T.reshape(D, NB, PB).transpose(1, 0, 2))
    xt = xTb.astype(BF16)
    if FP8_G1:
        xt8 = xTb.astype(FP8)
        w1_bf = (W1 * W1_SCALE).astype(FP8)
    else:
        w1_bf = W1.astype(BF16)                                    # [C, D, D]
    b1t = np.ascontiguousarray(
        b1.reshape(C, KT, 128).transpose(0, 2, 1)).astype(np.float32)
    w2t = np.ascontiguousarray(
        w2.reshape(C, KT, 128).transpose(0, 2, 1)).astype(np.float32)
    wl_pad = np.zeros((C, D, JP), dtype=BF16)
    wl_pad[:, :, :D + 1] = Wl[:, :D, :].astype(BF16)
    wlrow = np.zeros((C, JP), dtype=np.float32)
    wlrow[:, :D + 1] = Wl[:, D, :]
    wlrt = np.ascontiguousarray(wlrow.reshape(C, JT, 128).transpose(0, 2, 1))
    bl_pad = np.zeros((C, JP), dtype=np.float32)
    bl_pad[:, :D + 1] = bl
    blt = np.ascontiguousarray(bl_pad.reshape(C, JT, 128).transpose(0, 2, 1))
    wmu_t = np.ascontiguousarray(
        Wmu[:D].astype(BF16).reshape(KT, 128, E).transpose(1, 0, 2))
    wlv_t = np.ascontiguousarray(
        Wlv[:D].astype(BF16).reshape(KT, 128, E).transpose(1, 0, 2))
    wmur = np.ascontiguousarray(Wmu[D].reshape(E, 1)).astype(np.float32)
    wlvr = np.ascontiguousarray(Wlv[D].reshape(E, 1)).astype(np.float32)
    bmu2 = bmu.reshape(E, 1).astype(np.float32)
    blv2 = blv.reshape(E, 1).astype(np.float32)

    in_maps = []
    for c in range(N_CORES):
        sl = slice(c * CL, (c + 1) * CL)
        im = {
            "xt": xt,
            "w1": w1_bf[sl],
            "b1t": b1t[sl],
            "w2t": w2t[sl],
            "b2": np.ascontiguousarray(b2[sl].reshape(1, CL)),
            "wl": wl_pad[sl],
            "wlrt": wlrt[sl],
            "blt": blt[sl],
            "wmu": wmu_t,
            "bmu": bmu2,
            "wmur": wmur,
            "wlv": wlv_t,
            "blv": blv2,
            "wlvr": wlvr,
        }
        if FP8_G1:
            im["xt8"] = xt8
        in_maps.append(im)

    nc = _get_compiled()
    res = run_bass_kernel_spmd(nc, in_maps, core_ids=list(range(N_CORES)))
    kernel.last_results = res

    c_pred = np.concatenate([res.results[c]["c_pred"] for c in range(N_CORES)],
                            axis=0).T.astype(np.float32)            # [B, C]
    mu = np.concatenate([res.results[c]["mu"] for c in range(N_CORES)],
                        axis=0).transpose(2, 0, 1).astype(np.float32)  # [B,C,E]
    logvar = np.concatenate([res.results[c]["lv"] for c in range(N_CORES)],
                            axis=0).transpose(2, 0, 1).astype(np.float32)
    c_emb = mu.copy()
    return (np.ascontiguousarray(c_pred), c_emb, mu,
            np.ascontiguousarray(logvar))
